# revision 9
# baseline (speedup 1.0000x reference)
"""CRF loss kernel for Trainium2 (8 NeuronCores, data-parallel over batch).

Math: the log-domain forward recurrence
    alpha_t[i] = logsumexp_j(alpha_{t-1}[j] + trans[i,j]) + feat_t[i]
is run in probability domain:
    P_t = exp(feat_t - c) * (E @ P_{t-1}),   E = exp(trans)
so each step is one tiny 64x64 TensorE matmul plus one VectorE multiply.
A constant shift c plus an exact per-batch renorm every R steps keeps P in
f32 range. The STOP row of the matmul output doubles as the "partition
function if the sequence ended at step t" value, so no per-step masking by
seq_len is needed: the full P history row 63 is archived in SBUF and the
host epilogue picks slot seq_len[b]+1 per batch.
"""
import numpy as np

_B, _S, _T = 512, 512, 64
_NCORE = 8
_BC = _B // _NCORE          # 64 batches per core
_START, _STOP = 62, 63
_R = 32                     # renorm period
_NSTEP = _S + 1             # matmul steps 1..513
_NHIST = _NSTEP + 1         # history slots 0..513
_RENORM_T = list(range(_R, _S - 1, _R))   # 32,64,...,480
_NEVT = len(_RENORM_T)      # 15
_CH = 8                     # steps per feat DMA/exp chunk

_cache = {}


def _build_nc():
    import concourse.bass as bass
    import concourse.bacc as bacc
    import concourse.tile as tile
    from concourse import mybir
    from contextlib import ExitStack

    f32 = mybir.dt.float32
    nc = bacc.Bacc("TRN2", target_bir_lowering=False, debug=False,
                   num_devices=_NCORE)
    featT = nc.dram_tensor("featT", [_T, _NSTEP * _BC], f32, kind="ExternalInput").ap()
    # EP = [E.T | p0]: transition matrix and the initial state, one DMA so the
    # first matmul carries a single semaphore wait (LDWEIGHTS encodes only one).
    ep = nc.dram_tensor("ep", [_T, _T + _BC], f32, kind="ExternalInput").ap()
    hist63 = nc.dram_tensor("hist63", [1, _NHIST * _BC], f32, kind="ExternalOutput").ap()
    sinvh = nc.dram_tensor("sinvh", [1, max(_NEVT, 1) * _BC], f32, kind="ExternalOutput").ap()

    with tile.TileContext(nc) as tc, ExitStack() as ctx:
        consts = ctx.enter_context(tc.tile_pool(name="consts", bufs=1))
        fpool = ctx.enter_context(tc.tile_pool(name="fpool", bufs=3))
        epool = ctx.enter_context(tc.tile_pool(name="epool", bufs=3))
        pcur = ctx.enter_context(tc.tile_pool(name="pcur", bufs=2))
        ps_main = ctx.enter_context(tc.tile_pool(name="ps_main", bufs=3, space="PSUM"))
        ps_aux = ctx.enter_context(tc.tile_pool(name="ps_aux", bufs=2, space="PSUM"))

        ep_sb = consts.tile([_T, _T + _BC], f32)
        nc.sync.dma_start(ep_sb[:, :], ep)
        E_sb = ep_sb[:, 0:_T]
        ones_k = consts.tile([_T, 1], f32)
        nc.vector.memset(ones_k[:, :], 1.0)
        ones_m = consts.tile([1, _T], f32)
        nc.vector.memset(ones_m[:, :], 1.0)
        sinv_sb = consts.tile([1, max(_NEVT, 1) * _BC], f32)

        hist = consts.tile([_T, _NHIST * _BC], f32)

        cur_rhs = ep_sb[:, _T:_T + _BC]
        evt = 0
        t = 1
        while t <= _NSTEP:
            n_t = min(_CH, _NSTEP - t + 1)
            fchunk = fpool.tile([_T, _CH * _BC], f32, tag="fchunk")
            nc.sync.dma_start(
                fchunk[:, : n_t * _BC],
                featT[:, (t - 1) * _BC: (t - 1 + n_t) * _BC],
            )
            Fchunk = epool.tile([_T, _CH * _BC], f32, tag="Fchunk")
            nc.scalar.activation(
                Fchunk[:, : n_t * _BC], fchunk[:, : n_t * _BC],
                mybir.ActivationFunctionType.Exp,
            )
            for k in range(n_t):
                ps = ps_main.tile([_T, _BC], f32, tag="ps")
                nc.tensor.matmul(ps[:, :], E_sb[:, :], cur_rhs, start=True, stop=True)
                dst = hist[:, t * _BC: (t + 1) * _BC]
                nc.vector.tensor_mul(dst, ps[:, :], Fchunk[:, k * _BC: (k + 1) * _BC])
                cur_rhs = dst
                if t in _RENORM_T:
                    s_ps = ps_aux.tile([1, _BC], f32, tag="s_ps")
                    nc.tensor.matmul(s_ps[:, :], ones_k[:, :], dst, start=True, stop=True)
                    sv = sinv_sb[:, evt * _BC: (evt + 1) * _BC]
                    nc.vector.reciprocal(sv, s_ps[:, :])
                    bc = ps_aux.tile([_T, _BC], f32, tag="bc")
                    nc.tensor.matmul(bc[:, :], ones_m[:, :], sv, start=True, stop=True)
                    pc = pcur.tile([_T, _BC], f32, tag="pc")
                    nc.vector.tensor_mul(pc[:, :], dst, bc[:, :])
                    cur_rhs = pc[:, :]
                    evt += 1
                t += 1

        nc.sync.dma_start(hist63, hist[_STOP:_STOP + 1, :])
        nc.sync.dma_start(sinvh, sinv_sb[:, :])
    nc.compile()
    return nc


def _prep_inputs(feas, transitions):
    E = np.exp(transitions.astype(np.float32))
    rows = np.ones(_T, bool)
    rows[_START] = False
    c = float(np.log(E.sum(1)[rows]).mean())
    lhsT = np.ascontiguousarray(E.T).astype(np.float32)   # lhsT[j,i] = E[i,j]

    # featT per core: [T, NSTEP*BC]; featT[i, (t-1)*BC + b] = feas[b0+b, t-1, i] - c
    # for t<=S; step 513 gets feat=0 -> -c.
    ft = np.transpose(feas.astype(np.float32), (2, 1, 0)) - np.float32(c)  # [T,S,B]
    in_maps = []
    for cix in range(_NCORE):
        sl = ft[:, :, cix * _BC: (cix + 1) * _BC]                       # [T,S,BC]
        full = np.empty((_T, _NSTEP, _BC), np.float32)
        full[:, :_S, :] = sl
        full[:, _S, :] = -c
        p0 = np.zeros((_T, _BC), np.float32)
        p0[_START, :] = 1.0
        in_maps.append({
            "featT": np.ascontiguousarray(full.reshape(_T, _NSTEP * _BC)),
            "ep": np.ascontiguousarray(np.hstack([lhsT, p0])),
        })
    return c, in_maps


def kernel(feas, transitions, tag, seq_len):
    from concourse.bass_utils import run_bass_kernel_spmd

    feas = np.asarray(feas)
    transitions = np.asarray(transitions)
    tag = np.asarray(tag)
    seq_len = np.asarray(seq_len)

    if "nc" not in _cache:
        _cache["nc"] = _build_nc()
    nc = _cache["nc"]

    c, in_maps = _prep_inputs(feas, transitions)
    res = run_bass_kernel_spmd(nc, in_maps, list(range(_NCORE))).results

    # ---- host epilogue: norm from archived history ----
    L = seq_len.astype(np.int64)                                        # [B]
    hist63 = np.concatenate(
        [res[cix]["hist63"].reshape(_NHIST, _BC) for cix in range(_NCORE)], axis=1
    )                                                                   # [NHIST, B]
    logsum = np.zeros(_B, np.float64)
    for e, t in enumerate(_RENORM_T):
        sinv = np.concatenate(
            [res[cix]["sinvh"].reshape(_NEVT, _BC)[e] for cix in range(_NCORE)]
        )
        logsum += np.where(t <= L, -np.log(sinv.astype(np.float64)), 0.0)
    featT_val = np.where(
        L < _S,
        feas[np.arange(_B), np.minimum(L, _S - 1), _STOP].astype(np.float64) - c,
        -c,
    )
    norm = c * L + logsum + np.log(hist63[L + 1, np.arange(_B)].astype(np.float64)) - featT_val

    # ---- gold score ----
    dt = np.float32
    pos = np.arange(_S + 2)
    lbl = np.concatenate(
        [np.full((_B, 1), _START, tag.dtype), tag, np.full((_B, 1), _STOP, tag.dtype)],
        axis=1,
    )
    lbl = np.where(pos[None, :] <= L[:, None], lbl, _STOP)
    trn = transitions[lbl[:, 1:], lbl[:, :-1]]
    tmask = (np.arange(_S + 1)[None, :] <= L[:, None]).astype(dt)
    trans_score = (trn.astype(dt) * tmask).sum(1)
    emit = np.take_along_axis(feas, tag[..., None], axis=2)[..., 0]
    emask = (np.arange(_S)[None, :] < L[:, None]).astype(dt)
    emit_score = (emit.astype(dt) * emask).sum(1)

    return (norm - (trans_score + emit_score)).astype(np.float32)


# revision 12
# speedup vs baseline: 6845.2947x; 6845.2947x over previous
"""CRF loss kernel for Trainium2 (8 NeuronCores, data-parallel over batch).

Math: the log-domain forward recurrence
    alpha_t[i] = logsumexp_j(alpha_{t-1}[j] + trans[i,j]) + feat_t[i]
is run in probability domain:
    P_t = exp(feat_t - c) * (E @ P_{t-1}),   E = exp(trans)
so each step is one tiny 64x64 TensorE matmul plus one VectorE multiply.
A constant shift c plus a per-batch renorm every R steps keeps P in f32
range; the renorm scale is measured off the critical path and folded into
the exp(feat) tile of a later step, so the serial MM->mul chain never
grows extra links. The STOP row of each matmul output doubles as the
"partition function if the sequence ended here" value, so no per-step
seq_len masking is needed: row 63 of the full state history is archived
in SBUF and the host epilogue picks slot seq_len[b]+1 per batch.
"""
import numpy as np

_B, _S, _T = 512, 512, 64
_NCORE = 8
_BC = _B // _NCORE          # 64 batches per core
_START, _STOP = 62, 63
_R = 32                     # renorm period
_LAG = 4                    # renorm measured at t applies at t+_LAG
_NSTEP = _S + 1             # matmul steps 1..513
_NHIST = _NSTEP + 1         # history slots 0..513
_RENORM_T = list(range(_R, _S - 1, _R))   # 32,64,...,480
_NEVT = len(_RENORM_T)      # 15
_CH = 16                    # steps per feat DMA/exp chunk
_WARM_MM = True             # shadow matmuls to keep the PE HAM un-throttled

_cache = {}


def _build_nc():
    import concourse.bass as bass
    import concourse.bacc as bacc
    import concourse.tile as tile
    from concourse import mybir
    from contextlib import ExitStack

    f32 = mybir.dt.float32
    nc = bacc.Bacc("TRN2", target_bir_lowering=False, debug=False,
                   num_devices=_NCORE)
    featT = nc.dram_tensor("featT", [_T, _NSTEP * _BC], f32, kind="ExternalInput").ap()
    # EP = [E.T | p0]: transition matrix and the initial state, one DMA so the
    # first matmul carries a single semaphore wait (LDWEIGHTS encodes only one).
    ep = nc.dram_tensor("ep", [_T, _T + _BC], f32, kind="ExternalInput").ap()
    hist63 = nc.dram_tensor("hist63", [1, _NHIST * _BC], f32, kind="ExternalOutput").ap()
    sinvh = nc.dram_tensor("sinvh", [1, max(_NEVT, 1) * _BC], f32, kind="ExternalOutput").ap()

    with tile.TileContext(nc) as tc, ExitStack() as ctx:
        consts = ctx.enter_context(tc.tile_pool(name="consts", bufs=1))
        fpool = ctx.enter_context(tc.tile_pool(name="fpool", bufs=3))
        epool = ctx.enter_context(tc.tile_pool(name="epool", bufs=3))
        ps_main = ctx.enter_context(tc.tile_pool(name="ps_main", bufs=3, space="PSUM"))
        ps_aux = ctx.enter_context(tc.tile_pool(name="ps_aux", bufs=2, space="PSUM"))

        ep_sb = consts.tile([_T, _T + _BC], f32)
        nc.sync.dma_start(ep_sb[:, :], ep)
        E_sb = ep_sb[:, 0:_T]
        ones_k = consts.tile([_T, 1], f32)
        nc.vector.memset(ones_k[:, :], 1.0)
        ones_m = consts.tile([1, _T], f32)
        nc.vector.memset(ones_m[:, :], 1.0)
        sinv_sb = consts.tile([1, max(_NEVT, 1) * _BC], f32)

        hist = consts.tile([_T, _NHIST * _BC], f32)
        nc.vector.memset(hist[:, 0:_BC], 0.0)

        if _WARM_MM:
            ps_warm = ctx.enter_context(
                tc.tile_pool(name="ps_warm", bufs=1, space="PSUM"))
            warm_tile = ps_warm.tile([_T, _BC], f32)

        # F chunks: Fchunks[chunk_index] -> (tile, base_step). Renorm folds
        # multiply into a slice of a later chunk, so keep handles around.
        fold_at = {}                      # target step -> event index
        for e, t in enumerate(_RENORM_T):
            fold_at[t + _LAG] = e

        cur_rhs = ep_sb[:, _T:_T + _BC]
        evt_bc = {}                       # event index -> broadcast psum tile
        t = 1
        while t <= _NSTEP:
            n_t = min(_CH, _NSTEP - t + 1)
            fchunk = fpool.tile([_T, _CH * _BC], f32, tag="fchunk")
            nc.sync.dma_start(
                fchunk[:, : n_t * _BC],
                featT[:, (t - 1) * _BC: (t - 1 + n_t) * _BC],
            )
            Fchunk = epool.tile([_T, _CH * _BC], f32, tag="Fchunk")
            nc.scalar.activation(
                Fchunk[:, : n_t * _BC], fchunk[:, : n_t * _BC],
                mybir.ActivationFunctionType.Exp,
            )
            for k in range(n_t):
                if t in fold_at:
                    # apply the pending renorm scale to this step's F in place
                    bc = evt_bc.pop(fold_at[t])
                    fsl = Fchunk[:, k * _BC: (k + 1) * _BC]
                    nc.vector.tensor_mul(fsl, fsl, bc[:, :])
                ps = ps_main.tile([_T, _BC], f32, tag="ps")
                nc.tensor.matmul(ps[:, :], E_sb[:, :], cur_rhs, start=True, stop=True)
                if _WARM_MM:
                    nc.tensor.matmul(warm_tile[:, :], E_sb[:, :], cur_rhs,
                                     start=True, stop=True, skip_group_check=True)
                dst = hist[:, t * _BC: (t + 1) * _BC]
                nc.vector.tensor_mul(dst, ps[:, :], Fchunk[:, k * _BC: (k + 1) * _BC])
                cur_rhs = dst
                if t in _RENORM_T:
                    e = _RENORM_T.index(t)
                    s_ps = ps_aux.tile([1, _BC], f32, tag="s_ps")
                    nc.tensor.matmul(s_ps[:, :], ones_k[:, :], dst, start=True, stop=True)
                    sv = sinv_sb[:, e * _BC: (e + 1) * _BC]
                    nc.vector.reciprocal(sv, s_ps[:, :])
                    bc = ps_aux.tile([_T, _BC], f32, tag="bc")
                    nc.tensor.matmul(bc[:, :], ones_m[:, :], sv, start=True, stop=True)
                    evt_bc[e] = bc
                t += 1

        nc.sync.dma_start(hist63, hist[_STOP:_STOP + 1, :])
        nc.sync.dma_start(sinvh, sinv_sb[:, :])
    nc.compile()
    return nc


def _prep_inputs(feas, transitions):
    E = np.exp(transitions.astype(np.float32))
    rows = np.ones(_T, bool)
    rows[_START] = False
    c = float(np.log(E.sum(1)[rows]).mean())
    lhsT = np.ascontiguousarray(E.T).astype(np.float32)   # lhsT[j,i] = E[i,j]

    # featT per core: [T, NSTEP*BC]; featT[i, (t-1)*BC + b] = feas[b0+b, t-1, i] - c
    # for t<=S; step 513 gets feat=0 -> -c.
    ft = np.transpose(feas.astype(np.float32), (2, 1, 0)) - np.float32(c)  # [T,S,B]
    in_maps = []
    for cix in range(_NCORE):
        sl = ft[:, :, cix * _BC: (cix + 1) * _BC]                       # [T,S,BC]
        full = np.empty((_T, _NSTEP, _BC), np.float32)
        full[:, :_S, :] = sl
        full[:, _S, :] = -c
        p0 = np.zeros((_T, _BC), np.float32)
        p0[_START, :] = 1.0
        in_maps.append({
            "featT": np.ascontiguousarray(full.reshape(_T, _NSTEP * _BC)),
            "ep": np.ascontiguousarray(np.hstack([lhsT, p0])),
        })
    return c, in_maps


def kernel(feas, transitions, tag, seq_len):
    from concourse.bass_utils import run_bass_kernel_spmd

    feas = np.asarray(feas)
    transitions = np.asarray(transitions)
    tag = np.asarray(tag)
    seq_len = np.asarray(seq_len)

    if "nc" not in _cache:
        _cache["nc"] = _build_nc()
    nc = _cache["nc"]

    c, in_maps = _prep_inputs(feas, transitions)
    res = run_bass_kernel_spmd(nc, in_maps, list(range(_NCORE))).results

    # ---- host epilogue: norm from archived history ----
    L = seq_len.astype(np.int64)                                        # [B]
    hist63 = np.concatenate(
        [res[cix]["hist63"].reshape(_NHIST, _BC) for cix in range(_NCORE)], axis=1
    )                                                                   # [NHIST, B]
    logsum = np.zeros(_B, np.float64)
    for e, t in enumerate(_RENORM_T):
        sinv = np.concatenate(
            [res[cix]["sinvh"].reshape(_NEVT, _BC)[e] for cix in range(_NCORE)]
        )
        # scale 1/s_e is folded into F of step t+_LAG, so it is present in
        # hist slot m for m >= t+_LAG; capture slot is m = L+1.
        logsum += np.where(t + _LAG <= L + 1, -np.log(sinv.astype(np.float64)), 0.0)
    featT_val = np.where(
        L < _S,
        feas[np.arange(_B), np.minimum(L, _S - 1), _STOP].astype(np.float64) - c,
        -c,
    )
    norm = c * L + logsum + np.log(hist63[L + 1, np.arange(_B)].astype(np.float64)) - featT_val

    # ---- gold score ----
    dt = np.float32
    pos = np.arange(_S + 2)
    lbl = np.concatenate(
        [np.full((_B, 1), _START, tag.dtype), tag, np.full((_B, 1), _STOP, tag.dtype)],
        axis=1,
    )
    lbl = np.where(pos[None, :] <= L[:, None], lbl, _STOP)
    trn = transitions[lbl[:, 1:], lbl[:, :-1]]
    tmask = (np.arange(_S + 1)[None, :] <= L[:, None]).astype(dt)
    trans_score = (trn.astype(dt) * tmask).sum(1)
    emit = np.take_along_axis(feas, tag[..., None], axis=2)[..., 0]
    emask = (np.arange(_S)[None, :] < L[:, None]).astype(dt)
    emit_score = (emit.astype(dt) * emask).sum(1)

    return (norm - (trans_score + emit_score)).astype(np.float32)


# revision 14
# speedup vs baseline: 6876.3207x; 1.0045x over previous
"""CRF loss kernel for Trainium2 (8 NeuronCores, data-parallel over batch).

Math: the log-domain forward recurrence
    alpha_t[i] = logsumexp_j(alpha_{t-1}[j] + trans[i,j]) + feat_t[i]
is run in probability domain:
    P_t = exp(feat_t - c) * (E @ P_{t-1}),   E = exp(trans)
so each step is one tiny 64x64 TensorE matmul plus one VectorE multiply.
A constant shift c plus a per-batch renorm every R steps keeps P in f32
range; the renorm scale is measured off the critical path and folded into
the exp(feat) tile of a later step, so the serial MM->mul chain never
grows extra links. The STOP row of each matmul output doubles as the
"partition function if the sequence ended here" value, so no per-step
seq_len masking is needed: row 63 of the full state history is archived
in SBUF and the host epilogue picks slot seq_len[b]+1 per batch.
"""
import numpy as np

_B, _S, _T = 512, 512, 64
_NCORE = 8
_BC = _B // _NCORE          # 64 batches per core
_START, _STOP = 62, 63
_R = 32                     # renorm period
_LAG = 4                    # renorm measured at t applies at t+_LAG
_NSTEP = _S + 1             # matmul steps 1..513
_NHIST = _NSTEP + 1         # history slots 0..513
_RENORM_T = list(range(_R, _S - 1, _R))   # 32,64,...,480
_NEVT = len(_RENORM_T)      # 15
_CH = 8                     # steps per feat DMA/exp chunk
_WARM_MM = True             # shadow matmuls to keep the PE HAM un-throttled

_cache = {}


def _build_nc():
    import concourse.bass as bass
    import concourse.bacc as bacc
    import concourse.tile as tile
    from concourse import mybir
    from contextlib import ExitStack

    f32 = mybir.dt.float32
    nc = bacc.Bacc("TRN2", target_bir_lowering=False, debug=False,
                   num_devices=_NCORE)
    featT = nc.dram_tensor("featT", [_T, _NSTEP * _BC], f32, kind="ExternalInput").ap()
    # EP = [E.T | p0]: transition matrix and the initial state, one DMA so the
    # first matmul carries a single semaphore wait (LDWEIGHTS encodes only one).
    ep = nc.dram_tensor("ep", [_T, _T + _BC], f32, kind="ExternalInput").ap()
    hist63 = nc.dram_tensor("hist63", [1, _NHIST * _BC], f32, kind="ExternalOutput").ap()
    sinvh = nc.dram_tensor("sinvh", [1, max(_NEVT, 1) * _BC], f32, kind="ExternalOutput").ap()

    with tile.TileContext(nc) as tc, ExitStack() as ctx:
        consts = ctx.enter_context(tc.tile_pool(name="consts", bufs=1))
        fpool = ctx.enter_context(tc.tile_pool(name="fpool", bufs=3))
        epool = ctx.enter_context(tc.tile_pool(name="epool", bufs=3))
        ps_main = ctx.enter_context(tc.tile_pool(name="ps_main", bufs=3, space="PSUM"))
        ps_aux = ctx.enter_context(tc.tile_pool(name="ps_aux", bufs=2, space="PSUM"))

        ep_sb = consts.tile([_T, _T + _BC], f32)
        nc.sync.dma_start(ep_sb[:, :], ep)
        E_sb = ep_sb[:, 0:_T]
        ones_k = consts.tile([_T, 1], f32)
        nc.vector.memset(ones_k[:, :], 1.0)
        ones_m = consts.tile([1, _T], f32)
        nc.vector.memset(ones_m[:, :], 1.0)
        sinv_sb = consts.tile([1, max(_NEVT, 1) * _BC], f32)

        hist = consts.tile([_T, _NHIST * _BC], f32)
        nc.vector.memset(hist[:, 0:_BC], 0.0)

        if _WARM_MM:
            ps_warm = ctx.enter_context(
                tc.tile_pool(name="ps_warm", bufs=1, space="PSUM"))
            warm_tile = ps_warm.tile([_T, _BC], f32)

        # F chunks: Fchunks[chunk_index] -> (tile, base_step). Renorm folds
        # multiply into a slice of a later chunk, so keep handles around.
        fold_at = {}                      # target step -> event index
        for e, t in enumerate(_RENORM_T):
            fold_at[t + _LAG] = e

        cur_rhs = ep_sb[:, _T:_T + _BC]
        evt_bc = {}                       # event index -> broadcast psum tile
        t = 1
        while t <= _NSTEP:
            n_t = min(_CH, _NSTEP - t + 1)
            fchunk = fpool.tile([_T, _CH * _BC], f32, tag="fchunk")
            nc.sync.dma_start(
                fchunk[:, : n_t * _BC],
                featT[:, (t - 1) * _BC: (t - 1 + n_t) * _BC],
            )
            Fchunk = epool.tile([_T, _CH * _BC], f32, tag="Fchunk")
            nc.scalar.activation(
                Fchunk[:, : n_t * _BC], fchunk[:, : n_t * _BC],
                mybir.ActivationFunctionType.Exp,
            )
            for k in range(n_t):
                if t in fold_at:
                    # apply the pending renorm scale to this step's F in place
                    bc = evt_bc.pop(fold_at[t])
                    fsl = Fchunk[:, k * _BC: (k + 1) * _BC]
                    nc.vector.tensor_mul(fsl, fsl, bc[:, :])
                ps = ps_main.tile([_T, _BC], f32, tag="ps")
                nc.tensor.matmul(ps[:, :], E_sb[:, :], cur_rhs, start=True, stop=True)
                if _WARM_MM:
                    nc.tensor.matmul(warm_tile[:, :], E_sb[:, :], cur_rhs,
                                     start=True, stop=True, skip_group_check=True)
                dst = hist[:, t * _BC: (t + 1) * _BC]
                nc.vector.tensor_mul(dst, ps[:, :], Fchunk[:, k * _BC: (k + 1) * _BC])
                cur_rhs = dst
                if t in _RENORM_T:
                    e = _RENORM_T.index(t)
                    s_ps = ps_aux.tile([1, _BC], f32, tag="s_ps")
                    nc.tensor.matmul(s_ps[:, :], ones_k[:, :], dst, start=True, stop=True)
                    sv = sinv_sb[:, e * _BC: (e + 1) * _BC]
                    nc.vector.reciprocal(sv, s_ps[:, :])
                    bc = ps_aux.tile([_T, _BC], f32, tag="bc")
                    nc.tensor.matmul(bc[:, :], ones_m[:, :], sv, start=True, stop=True)
                    evt_bc[e] = bc
                t += 1

        nc.sync.dma_start(hist63, hist[_STOP:_STOP + 1, :])
        nc.sync.dma_start(sinvh, sinv_sb[:, :])
    nc.compile()
    return nc


def _prep_inputs(feas, transitions):
    E = np.exp(transitions.astype(np.float32))
    rows = np.ones(_T, bool)
    rows[_START] = False
    c = float(np.log(E.sum(1)[rows]).mean())
    lhsT = np.ascontiguousarray(E.T).astype(np.float32)   # lhsT[j,i] = E[i,j]

    # featT per core: [T, NSTEP*BC]; featT[i, (t-1)*BC + b] = feas[b0+b, t-1, i] - c
    # for t<=S; step 513 gets feat=0 -> -c.
    ft = np.transpose(feas.astype(np.float32), (2, 1, 0)) - np.float32(c)  # [T,S,B]
    in_maps = []
    for cix in range(_NCORE):
        sl = ft[:, :, cix * _BC: (cix + 1) * _BC]                       # [T,S,BC]
        full = np.empty((_T, _NSTEP, _BC), np.float32)
        full[:, :_S, :] = sl
        full[:, _S, :] = -c
        p0 = np.zeros((_T, _BC), np.float32)
        p0[_START, :] = 1.0
        in_maps.append({
            "featT": np.ascontiguousarray(full.reshape(_T, _NSTEP * _BC)),
            "ep": np.ascontiguousarray(np.hstack([lhsT, p0])),
        })
    return c, in_maps


def kernel(feas, transitions, tag, seq_len):
    from concourse.bass_utils import run_bass_kernel_spmd

    feas = np.asarray(feas)
    transitions = np.asarray(transitions)
    tag = np.asarray(tag)
    seq_len = np.asarray(seq_len)

    if "nc" not in _cache:
        _cache["nc"] = _build_nc()
    nc = _cache["nc"]

    c, in_maps = _prep_inputs(feas, transitions)
    res = run_bass_kernel_spmd(nc, in_maps, list(range(_NCORE))).results

    # ---- host epilogue: norm from archived history ----
    L = seq_len.astype(np.int64)                                        # [B]
    hist63 = np.concatenate(
        [res[cix]["hist63"].reshape(_NHIST, _BC) for cix in range(_NCORE)], axis=1
    )                                                                   # [NHIST, B]
    logsum = np.zeros(_B, np.float64)
    for e, t in enumerate(_RENORM_T):
        sinv = np.concatenate(
            [res[cix]["sinvh"].reshape(_NEVT, _BC)[e] for cix in range(_NCORE)]
        )
        # scale 1/s_e is folded into F of step t+_LAG, so it is present in
        # hist slot m for m >= t+_LAG; capture slot is m = L+1.
        logsum += np.where(t + _LAG <= L + 1, -np.log(sinv.astype(np.float64)), 0.0)
    featT_val = np.where(
        L < _S,
        feas[np.arange(_B), np.minimum(L, _S - 1), _STOP].astype(np.float64) - c,
        -c,
    )
    norm = c * L + logsum + np.log(hist63[L + 1, np.arange(_B)].astype(np.float64)) - featT_val

    # ---- gold score ----
    dt = np.float32
    pos = np.arange(_S + 2)
    lbl = np.concatenate(
        [np.full((_B, 1), _START, tag.dtype), tag, np.full((_B, 1), _STOP, tag.dtype)],
        axis=1,
    )
    lbl = np.where(pos[None, :] <= L[:, None], lbl, _STOP)
    trn = transitions[lbl[:, 1:], lbl[:, :-1]]
    tmask = (np.arange(_S + 1)[None, :] <= L[:, None]).astype(dt)
    trans_score = (trn.astype(dt) * tmask).sum(1)
    emit = np.take_along_axis(feas, tag[..., None], axis=2)[..., 0]
    emask = (np.arange(_S)[None, :] < L[:, None]).astype(dt)
    emit_score = (emit.astype(dt) * emask).sum(1)

    return (norm - (trans_score + emit_score)).astype(np.float32)
